# revision 77
# baseline (speedup 1.0000x reference)
"""CIEDE-base color-difference loss kernel for 8 Trainium2 NeuronCores.

Math (lightness_weight = 0, so L never matters):
  lin  = srgb_gamma(x)            -- ACT Ln+Exp, pow branch only; the linear
                                     branch is skipped (~1e-4 rel impact)
  t    = (RGB2XYZ/white) @ lin    -- PE block-diag float32r matmul
  f    = cbrt(t) = exp(ln(t)/3)   -- ACT Ln (from PSUM) + Exp; bf16 out
  da   = 500*((fx1-fy1)-(fx2-fy2));  db = 200*((fy1-fz1)-(fy2-fz2))  -- PE
  s    = da^2 + db^2              -- DVE squares (bf16) + PE pair-sum, packed
                                     at partition rows 21*lg of [126,1024]x2
                                     PSUM accumulators via shifted W3 blocks
  cd   = sqrt(s)                  -- single-pass AF.Sqrt at the very end
  out  = mean over pixels         -- ACT accum_out + host-side f64 reduce

Layout: batch data-parallel, 4 image-pairs per core. Each image plane is
host-padded to 21 partition-rows x 12544 cols (pad = 0.5 in both images so
padded pixels contribute cd ~ 0). Partitions are channel-blocked (p = 21*c+k,
c in r1,g1,b1,r2,g2,b2 -- host packs both images per pair contiguously), so
each [126, F] tile holds 21*F pixel pairs in ONE 2-dim DMA, and every
cross-channel op is a kron(A, I21) matmul on the mostly-idle PE.

Schedule notes (ACT is the bottleneck engine; sim has it ~96% busy):
 - activation-table thrash fix: all ln/exp resolve to one table set (see
   _build_module), saving ~72 reloads (~92us, 28% of the original runtime)
 - 4096-col tiles (2 logical 2048 groups) amortize ACT instruction overhead;
   the first tile's DMA+Ln are eighth-split so ACT starts at ~4us
 - one-step software pipeline: tile t's gamma is emitted before tile t-1's
   cbrt/stage-2/3, so the f32r stage-1 matmuls (1.2GHz) get a full gamma
   pass of head start and the cbrt-Ln chunk reads never starve
 - all sqrts run at the end as single-pass AF.Sqrt behind ONE table switch,
   reading SBUF-parked bf16 s (pairs 0-2) or PSUM directly (pair 3); a
   zero-valued token written by a Relu(scale=0) of the last f tile is used
   as every Sqrt's bias AP, pinning ln/exp before sqrt in the ACT order
   without any cross-engine barrier
 - the pairs' ragged 256-col tails are batched into one [126,1024] chain
   processed LAST (short drain), its input DMA'd late in the stream
 - tile (1,1)'s gamma runs as a deg-8 fused-Horner polynomial on the DVE
   (10 ops, (y+c)*z scalar_tensor_tensor form, ~44us of idle-DVE time for
   ~7.2us of ACT; max abs err 5.3e-6, HW-validated by probe2.py), its ops
   interleaved 2-per-step so stage-2/3 DVE work never starves
Measured on HW (axon): rel_err 7.5e-5; TimelineSim (which matched the
baseline's graded time to 0.4%): 210,469 ns vs 328,174 ns baseline.
"""

import numpy as np
import ml_dtypes

B, C, H, W = 32, 3, 512, 512
HWPX = H * W                 # 262144 pixels per image
N_CORES = 8
B_LOC = B // N_CORES         # 4 image-pairs per core
ROWS = 21                    # partition-rows per image
ROWL = 12544                 # cols per partition-row; host pads each plane to
PADPX = ROWS * ROWL          # 263424 px (+1280 pad px, value 0.5 both images)
FULL_F = 2048                # cols per logical group (spack packing unit)
N_FULL = 6                   # logical groups per image pair (6*2048 = 12288)
TILE_F = 4096                # cols per physical x tile (2 logical groups):
N_TILE = 3                   # fatter ACT instructions amortize fixed overhead
RAG_F = ROWL - N_FULL * FULL_F   # 256 ragged cols

# deg-8 fit of ((x+0.055)/1.055)**2.4 on x in [0,1], evaluated in z = 2x-1
# (f32 fused-Horner max abs err 5.3e-6; used by the DVE-chain gamma path,
# whose exact op sequence is HW-validated bit-for-bit by probe2.py)
_POLY_Z = [0.2140408754348755, 0.4627854526042938, 0.2918641269207001,
           0.03514162823557854, -0.004844842944294214, 0.0010735071264207363,
           -0.0002527319302316755, 0.0005788140115328133,
           -0.0003889543586410582]

_RGB2XYZ = np.array([[0.4124564, 0.3575761, 0.1804375],
                     [0.2126729, 0.7151522, 0.0721750],
                     [0.0193339, 0.1191920, 0.9503041]], dtype=np.float64)
_WHITE = np.array([0.95047, 1.0, 1.08883], dtype=np.float64)

bf16 = ml_dtypes.bfloat16


def _build_weights():
    """Channel-blocked layout: partition p = 21*c + k (c = channel slot 0..5
    meaning r1,g1,b1,r2,g2,b2; k = pixel row 0..20). All block-diagonal maps
    become kron(A, I21)."""
    f32 = np.float32
    I21 = np.eye(ROWS, dtype=f32)
    Mp = (_RGB2XYZ / _WHITE[:, None]).astype(f32)          # 3x3, white folded in
    M6 = np.zeros((6, 6), f32)
    M6[:3, :3] = Mp
    M6[3:, 3:] = Mp
    # stage 1: out = lhsT.T @ rhs; want out[21c'+k] = sum_c M6[c',c] v[21c+k]
    # Used as float32r (full-precision weights)
    W1 = np.kron(M6.T, I21).astype(f32)                     # [126, 126]
    # stage 2: da rows 0..20, db rows 21..41; coeffs exact in bf16
    A = np.array([[500.0, -500.0, 0.0, -500.0, 500.0, 0.0],
                  [0.0, 200.0, -200.0, 0.0, -200.0, 200.0]], f32)
    W2 = np.kron(A.T, I21).astype(f32)                      # [126, 42]
    # stage 3: s[k] = sq[k] + sq[21+k]. For full groups the result is placed
    # at partition rows 21*gg of a [126, F] PSUM accumulator: W3all[:, 126g+p]
    # has the summing I21 block at rows offset 21*g. The cross-pair ragged
    # batch uses B_LOC variants [42, 84] that place pair b's sums at rows 21b.
    W3 = np.kron(np.ones((2, 1), f32), I21).astype(f32)     # [42, 21]
    W3all = np.zeros((42, 126 * N_FULL), f32)
    for g in range(N_FULL):
        W3all[:, 126 * g + 21 * g: 126 * g + 21 * g + ROWS] = W3
    W3rag = np.zeros((42, 84 * B_LOC), f32)
    for b in range(B_LOC):
        W3rag[:, 84 * b + 21 * b: 84 * b + 21 * b + ROWS] = W3
    return W1, W2.astype(bf16), W3all.astype(bf16), W3rag.astype(bf16)


_CACHE = {}


def _build_module(reps=1, variant="full"):
    """variant: 'full' | 'dma' (loads + tiny DVE reduce) | 'dma_act' (loads +
    2 big ACT passes, no PE/DVE pipeline)."""
    import concourse.bass as bass
    import concourse.bacc as bacc
    import concourse.tile as tile
    from concourse import mybir

    # The act-table-load placement pass picks the FIRST table set containing
    # each activation function: Exp -> exp_and_others (id 0, no ln), Ln ->
    # natural_log (id 5, no exp). Alternating Ln/Exp then reloads tables
    # ~72x (~1.3us each, ~92us = 28% of runtime). Hide ln/exp in every set
    # except natural_log_exp_and_others (which genuinely holds both) so the
    # analysis settles on that one set and the load hoists out of the loop.
    # Only placement is affected; the emitted set id stays a valid
    # act_info.json index whose tables contain every function we use.
    if not getattr(bacc, "_lnexp_tables_patch", False):
        _orig_gat = bacc.get_activation_tables

        def _gat_one_set(arch):
            AF = mybir.ActivationFunctionType
            out = {}
            for name, funcs in _orig_gat(arch).items():
                if name != "natural_log_exp_and_others":
                    funcs = funcs - {AF.Ln, AF.Exp}
                out[name] = funcs
            return out

        bacc.get_activation_tables = _gat_one_set
        bacc._lnexp_tables_patch = True

    f32 = mybir.dt.float32
    bft = mybir.dt.bfloat16
    AF = mybir.ActivationFunctionType

    nc = bacc.Bacc(None, target_bir_lowering=False)

    # img1/img2 host-packed per pair: [pair, img, ch, px] makes the full
    # 126-partition block (2*3*21 rows) one uniformly-strided dim, so a
    # [126, F] tile is ONE 2-dim DMA -- halves the HWDGE issue serialization
    imgsh = nc.dram_tensor("imgs", [B_LOC, 2, C, PADPX], f32,
                           kind="ExternalInput")
    f32r = mybir.dt.float32r
    w1h = nc.dram_tensor("w1", [126, 126], f32r, kind="ExternalInput")
    w1fh = nc.dram_tensor("w1f32", [126, 126], f32, kind="ExternalInput")
    w2h = nc.dram_tensor("w2", [126, 42], bft, kind="ExternalInput")
    w3ah = nc.dram_tensor("w3all", [42, 126 * N_FULL], bft, kind="ExternalInput")
    w3rh = nc.dram_tensor("w3rag", [42, 84 * B_LOC], bft, kind="ExternalInput")
    outh = nc.dram_tensor("partials", [126, B_LOC + 3], f32, kind="ExternalOutput")

    # ln((x+0.055)/1.055) then exp(2.4*l)
    GAMMA_SCALE = float(1.0 / 1.055)
    GAMMA_BIAS = float(0.055 / 1.055)

    def dram_src6(b, col0, ncols):
        """[2 imgs x 3 channels x 21 rows, ncols] of pair b: one DMA."""
        off = b * 2 * C * PADPX + col0
        return bass.AP(tensor=imgsh, offset=off,
                       ap=[[ROWL, 126], [1, ncols]])

    def dram_src_rag():
        """All pairs' trailing RAG_F cols: [126, B_LOC*RAG_F], pair-major
        cols, in one DMA."""
        return bass.AP(tensor=imgsh, offset=N_FULL * FULL_F,
                       ap=[[ROWL, 126], [2 * C * PADPX, B_LOC], [1, RAG_F]])

    with tile.TileContext(nc) as tc:
        from contextlib import ExitStack
        with ExitStack() as ctx:
            singles = ctx.enter_context(tc.tile_pool(name="singles", bufs=1))
            xpool = ctx.enter_context(tc.tile_pool(name="x", bufs=2))
            linpool = ctx.enter_context(tc.tile_pool(name="lin", bufs=4))
            ltpool = ctx.enter_context(tc.tile_pool(name="lt", bufs=1))
            fpool = ctx.enter_context(tc.tile_pool(name="f", bufs=2))
            sqpool = ctx.enter_context(tc.tile_pool(name="sq", bufs=2))
            ddcpool = ctx.enter_context(tc.tile_pool(name="ddc", bufs=2))
            qpool = ctx.enter_context(tc.tile_pool(name="q", bufs=1))
            # PSUM bank budget (8 banks): t [126,512]x3 = 3, dd [42,512] = 1,
            # s [126,1024]x2x2 = 4 (two half-accumulators; srag shares dd)
            tpool = ctx.enter_context(tc.tile_pool(name="t", bufs=3, space="PSUM"))
            ddpool = ctx.enter_context(tc.tile_pool(name="dd", bufs=1, space="PSUM"))
            spool = ctx.enter_context(tc.tile_pool(name="s", bufs=2, space="PSUM"))

            w1f = singles.tile([126, 126], f32r)
            w1p = singles.tile([126, 126], f32)
            w2 = singles.tile([126, 42], bft)
            w3a = singles.tile([42, 126 * N_FULL], bft)
            w3r = singles.tile([42, 84 * B_LOC], bft)

            # DMA transfers serialize at ~350GB/s in issue order, so issue
            # exactly what the pipeline needs first: the first tile's
            # quarter-splits (emitted in the pair loop below), then weights,
            # then the ragged batch input (xr: computed LAST, parked in a
            # static slot so xpool keeps all its lookahead).
            FR = B_LOC * RAG_F
            xr = singles.tile([126, FR], f32)

            def load_statics():
                nc.sync.dma_start(out=w1f[:], in_=w1h[:, :])
                nc.sync.dma_start(out=w1p[:], in_=w1fh[:, :])
                nc.sync.dma_start(out=w2[:], in_=w2h[:, :])
                nc.sync.dma_start(out=w3a[:], in_=w3ah[:, :])
                nc.sync.dma_start(out=w3r[:], in_=w3rh[:, :])

            # acc cols: 0..2 pairs 0-2, 3/4 pair-3 halves (accum_out overwrites,
            # so the two PSUM-read Sqrts need distinct columns), 5 ragged
            acc = singles.tile([126, B_LOC + 3], f32)
            nc.vector.memset(acc[:], 0.0)

            gbias = singles.tile([128, 1], f32)
            nc.vector.memset(gbias[:], GAMMA_BIAS)
            ebias = singles.tile([128, 1], f32)
            nc.vector.memset(ebias[:], 1e-35)

            # 1-col warmup: hoists the ln/exp table load to t~0.4us; without
            # it the load sits behind the first gamma Ln's DMA-wait (~4us)
            warm = singles.tile([128, 1], f32)
            nc.scalar.activation(out=warm[:], in_=gbias[:], func=AF.Exp)

            # static input/output for the DVE-chain gamma of tile (1,1):
            # z in place on zdve, fused Horner accumulates in lindve
            zdve = singles.tile([126, TILE_F], f32)
            lindve = singles.tile([126, TILE_F], f32r)

            def make_dve_chain():
                """10 closures, each one DVE op of the deg-8 fused Horner
                gamma for tile (1,1); popped 2 per pipeline step so the
                in-order DVE queue never blocks stage-2/3 work for long.
                Replaces the tile's ACT Ln+Exp (~7.2us of the bottleneck
                engine) with ~44us of otherwise-idle DVE time."""
                mul = mybir.AluOpType.mult
                add = mybir.AluOpType.add
                ops = [lambda: nc.vector.tensor_scalar(
                    out=zdve[:], in0=zdve[:], scalar1=2.0, scalar2=-1.0,
                    op0=mul, op1=add)]
                ops.append(lambda: nc.vector.tensor_scalar(
                    out=lindve[:], in0=zdve[:], scalar1=float(_POLY_Z[8]),
                    scalar2=None, op0=mul))
                for kk in range(7, 0, -1):
                    ops.append(lambda kk=kk: nc.vector.scalar_tensor_tensor(
                        out=lindve[:], in0=lindve[:],
                        scalar=float(_POLY_Z[kk]),
                        in1=zdve[:], op0=add, op1=mul))
                ops.append(lambda: nc.vector.tensor_scalar(
                    out=lindve[:], in0=lindve[:], scalar1=float(_POLY_Z[0]),
                    scalar2=None, op0=add))
                return ops


            if reps > 1:
                loop_cm = tc.For_i(0, reps, 1)
                loop_cm.__enter__()

            def gamma(x, F, split_ln=False, ln_dst=None):
                """gamma on a [126, F] tile; returns lin (ACT Ln+Exp only).

                ln_dst: where the gamma Ln lands (default: in place on x).
                The reps>1 timing loop re-runs the body on the SAME parked
                ragged tile, so that caller must keep x pristine -- ln of an
                already-ln'd (negative) value is NaN, and NaN operands are
                catastrophically slow on real hardware."""
                if ln_dst is None:
                    ln_dst = x
                # l = ln((x+0.055)/1.055), lin = exp(2.4 l)
                if split_ln:
                    # per-eighth Ln tracks the eighth-split first DMA
                    for qq in range(F // 512):
                        nc.scalar.activation(
                            out=ln_dst[:, qq*512:qq*512+512],
                            in_=x[:, qq*512:qq*512+512], func=AF.Ln,
                            scale=GAMMA_SCALE, bias=gbias[0:126])
                else:
                    nc.scalar.activation(out=ln_dst[:], in_=x[:], func=AF.Ln,
                                         scale=GAMMA_SCALE, bias=gbias[0:126])
                lin = linpool.tile([126, F], f32r, tag="lin")
                nc.scalar.activation(out=lin[:], in_=ln_dst[:], func=AF.Exp,
                                     scale=2.4)
                return lin

            def cbrt_evac(lin, F, w1sel=None):
                """XYZ stage-1 matmuls + cbrt for a lin tile; returns f.

                Runs one pipeline step behind gamma(): the stage-1 f32r
                matmuls (788ns/512-chunk at the PE's 1.2GHz fp32 rate) get a
                full gamma-pass head start, so the 612ns cbrt-Ln chunk reads
                never drain the 3-buffer tq pool dry."""
                lt = ltpool.tile([126, F], f32, tag="lt")
                for h in range((F + 511) // 512):
                    c0 = h * 512
                    cw = min(512, F - c0)
                    tq = tpool.tile([126, cw], f32, tag="t")
                    w1ap = w1sel if w1sel is not None else w1f[:]
                    nc.tensor.matmul(tq[:], w1ap, lin[:, c0:c0+cw],
                                     start=True, stop=True)
                    # cbrt part 1: lt = ln(t) straight from PSUM
                    nc.scalar.activation(out=lt[:, c0:c0+cw], in_=tq[:],
                                         func=AF.Ln)
                # cbrt part 2: f = exp(lt/3) as bf16
                f = fpool.tile([126, F], bft, tag="f")
                nc.scalar.activation(out=f[:], in_=lt[:], func=AF.Exp,
                                     scale=float(1.0 / 3.0))
                return f

            def stage2_sq(f, F):
                """da,db + squares for a [126, F] f tile; returns sq [42, F]."""
                sq = sqpool.tile([42, F], bft, tag="sq")
                for j in range((F + 511) // 512):
                    s0 = j * 512
                    sw = min(512, F - s0)
                    dd = ddpool.tile([42, sw], f32, tag="dd")
                    nc.tensor.matmul(dd[:], w2[:], f[:, s0:s0+sw],
                                     start=True, stop=True)
                    # DVE tensor_tensor may read only one PSUM operand:
                    # bounce da/db to SBUF bf16, square there (2x mode)
                    ddc = ddcpool.tile([42, sw], bft, tag="ddc")
                    nc.vector.tensor_copy(ddc[:], dd[:])
                    nc.vector.tensor_mul(sq[:, s0:s0+sw], ddc[:], ddc[:])
                return sq

            # ---- main pairs, 6 full groups each. Each pair's packed s is
            # parked to SBUF as bf16 by DVE (idle capacity) right after its
            # last stage-3 matmul; ALL sqrts run at the very end as single-
            # pass AF.Sqrt instructions behind one table switch, instead of
            # per-pair exp(0.5*ln(s)) chains (~7.5us less ACT busy).
            parks = []
            spacks = {}

            def get_spack(b):
                # lazily created by the FIRST s23 of pair b, so spool slot
                # allocation order matches actual write order under the
                # one-step software pipeline
                if b not in spacks:
                    # Two [126, 1024] PSUM accumulators (chunk-halves): rows
                    # 21*lg collect logical group lg's pair-sums via the
                    # shifted W3all blocks (PE writes all 126 partitions;
                    # non-block rows add zero)
                    spacks[b] = [spool.tile([126, FULL_F // 2], f32, tag="s",
                                            name=f"spack{b}_{_h}")
                                 for _h in range(2)]
                return spacks[b]

            s23_done = {}

            def s23(f, F, tt, b, stop_lg):
                """stage 2/3 + (after the pair's last tile) park/collect.
                stop_lg: the logical group accumulated LAST in time for this
                pair (3 when the pool tile's evac is deferred, else 5)."""
                sq = stage2_sq(f, F)
                spack = get_spack(b)
                for j in range(F // 512):
                    s0 = j * 512
                    lg = 2 * tt + (j >= 4)       # logical 2048-col group
                    half, hj = (j % 4) // 2, j % 2
                    nc.tensor.matmul(
                        spack[half][:, hj*512:hj*512+512],
                        w3a[:, 126*lg:126*lg+126],
                        sq[:, s0:s0+512],
                        start=(lg == 0),
                        stop=(lg == stop_lg))
                s23_done[b] = s23_done.get(b, 0) + 1
                if s23_done[b] == N_TILE:
                    if b < B_LOC - 1:
                        # park packed s to SBUF bf16 for the end-of-kernel
                        # Sqrt batch
                        park = singles.tile([126, FULL_F], bft,
                                            name=f"park{b}")
                        for half in range(2):
                            nc.vector.tensor_copy(
                                park[:, half*1024:half*1024+1024],
                                spack[half][:])
                        parks.append(("sbuf", park, b))
                    else:
                        parks.append(("psum", spack, b))

            # ---- main pairs, 3 tiles (6 logical groups) each, software-
            # pipelined one step: tile t's gamma is emitted BEFORE tile
            # t-1's cbrt/stage-2/3, giving the PE a full gamma-pass head
            # start on the stage-1 matmuls. Each pair's packed s is parked
            # to SBUF bf16 by DVE right after its last stage-3 matmul; ALL
            # sqrts run at the very end as single-pass AF.Sqrt instructions
            # behind one table switch (~7.5us less ACT busy than per-pair
            # exp(0.5*ln(s)) chains).
            pending = []
            dve_ops = make_dve_chain()

            def flush_pending(kind=None):
                for ent in list(pending):
                    if kind is not None and ent[-2] != kind:
                        continue
                    ent[-1] -= 1
                    if ent[-1] <= 0:
                        p_lin, p_F, p_tt, p_b, p_stop = ent[:5]
                        s23(cbrt_evac(p_lin, p_F), p_F, p_tt, p_b, p_stop)
                        pending.remove(ent)

            for b in range(B_LOC):
                for tt in range(N_TILE):
                    F = TILE_F
                    col0 = tt * TILE_F

                    k = 3 * b + tt
                    if variant == "full" and k == 4:
                        # tile (1,1) is covered by the DVE chain: no x load,
                        # no ACT gamma; run the remaining chain ops and let
                        # the pipeline flush (chain evac comes at k=5, AFTER
                        # (1,0)'s s23 -- flag order must stay lg 0,1/2,3/4,5
                        # or the start=True of a later-emitted lg0 matmul
                        # wipes the accumulation)
                        flush_pending()
                        for op in dve_ops:
                            op()
                        dve_ops = []
                        continue
                    x = xpool.tile([126, F], f32, tag="x")
                    if b == 0 and tt == 0:
                        # eighth-split the very first load: 512-col transfers
                        # (717ns) pace 512-col Lns (612ns) almost 1:1, so the
                        # pipeline starts after 0.25MB with no startup bubbles
                        for qq in range(8):
                            nc.sync.dma_start(
                                out=x[:, qq*512:qq*512+512],
                                in_=dram_src6(b, col0 + qq*512, 512))
                        load_statics()
                    else:
                        nc.sync.dma_start(out=x[:], in_=dram_src6(b, col0, F))

                    if variant == "dma":
                        red = qpool.tile([126, 1], f32, tag="red")
                        nc.vector.tensor_reduce(
                            out=red[:], in_=x[:], op=mybir.AluOpType.max,
                            axis=mybir.AxisListType.X)
                        continue  # noqa
                    if variant == "dma_act":
                        nc.scalar.activation(out=x[:], in_=x[:], func=AF.Ln,
                                             scale=GAMMA_SCALE, bias=gbias[0:126])
                        nc.scalar.activation(out=x[:], in_=x[:], func=AF.Exp,
                                             scale=2.4)
                        nc.scalar.activation(out=x[:], in_=x[:], func=AF.Ln,
                                             scale=GAMMA_SCALE, bias=gbias[0:126])
                        nc.scalar.activation(out=x[:], in_=x[:], func=AF.Exp,
                                             scale=float(1.0/3.0),
                                             accum_out=acc[0:126, b:b+1])
                        continue

                    if b == B_LOC - 1 and tt == 0:
                        # ragged batch input: needed only at the very end,
                        # so it rides late in the DMA stream where it can't
                        # delay any group tile
                        nc.sync.dma_start(out=xr[:], in_=dram_src_rag())
                    if variant == "full" and k == 1:
                        # tile (1,1)'s data, right after x(0,1) in the sync
                        # stream: a 2MB transfer any earlier steals the
                        # bandwidth the startup eighth-splits depend on.
                        # Emitted before the first chain op (popped below)
                        # so the op orders after the write. Counter 5:
                        # this iteration's flush already decrements it, so
                        # the evac lands at k=5, after (1,0)'s s23.
                        nc.sync.dma_start(out=zdve[:],
                                          in_=dram_src6(1, TILE_F, TILE_F))
                        pending.append([lindve, TILE_F, 1, 1, 5, "main", 5])
                    lin = gamma(x, F, split_ln=(b == 0 and tt == 0))
                    flush_pending()
                    pending.append([lin, F, tt, b, 5, "main", 1])
                    if variant == "full" and dve_ops and k >= 1:
                        for op in dve_ops[:2]:
                            op()
                        dve_ops = dve_ops[2:]

            # ---- cross-pair ragged batch: the pairs' trailing RAG_F cols,
            # packed as [126, B_LOC*RAG_F] (pair b at cols b*RAG_F). Processed
            # LAST: its 2-chunk pipeline drain + [84,256] sqrt make a ~3us
            # tail, vs ~7us for a full tile's 8-chunk drain. The last main
            # tile's cbrt + pair-3 sqrt hide inside the ragged chain.
            if variant == "dma":
                red = qpool.tile([126, 1], f32, tag="red")
                nc.vector.tensor_reduce(
                    out=red[:], in_=xr[:], op=mybir.AluOpType.max,
                    axis=mybir.AxisListType.X)
            elif variant == "dma_act":
                xl = xpool.tile([126, FR], f32, tag="x")
                nc.scalar.activation(out=xl[:], in_=xr[:], func=AF.Ln,
                                     scale=GAMMA_SCALE, bias=gbias[0:126])
                nc.scalar.activation(out=xl[:], in_=xl[:], func=AF.Exp,
                                     scale=2.4,
                                     accum_out=acc[0:126, B_LOC+2:B_LOC+3])
            else:
                xl = xpool.tile([126, FR], f32, tag="x")
                # flush the last main tile BEFORE the ragged gamma: its
                # square/stage-3 drain then overlaps the ragged ACT chain
                while pending:
                    flush_pending()
                lin_r = gamma(xr, FR, ln_dst=xl)
                fr = cbrt_evac(lin_r, FR)
                # ACT-order pin without a barrier: tok = relu(0*f_rag) = 0,
                # exactly zero but data-dependent on the LAST exp; every
                # Sqrt below uses tok as its bias AP (they read a const-0
                # bias AP anyway), so no sqrt can be scheduled before any
                # ln/exp -- one table switch, and no cross-engine gating
                tok = singles.tile([126, 1], f32)
                nc.scalar.activation(out=tok[:], in_=fr[:, 0:1],
                                     func=AF.Relu, scale=0.0)
                sqr = stage2_sq(fr, FR)
                srag = ddpool.tile([84, RAG_F], f32, tag="dd")
                for b in range(B_LOC):
                    nc.tensor.matmul(srag[:], w3r[:, 84*b:84*b+84],
                                     sqr[:, b*RAG_F:(b+1)*RAG_F],
                                     start=(b == 0), stop=(b == B_LOC - 1))

                # ---- all sqrts, single-pass AF.Sqrt behind one table switch.
                # Pairs 0..2 read parked SBUF bf16; pair 3 + ragged read PSUM.
                # The scheduler-only fence keeps every ln/exp before every
                # sqrt in the final ACT order: without it the list scheduler
                # interleaves them and the table-load pass inserts 2 extra
                # ~1.3us reloads.
                for kind, src, b in parks:
                    q = qpool.tile([126, FULL_F], bft, tag="qpack")
                    if kind == "sbuf":
                        nc.scalar.activation(out=q[:], in_=src[:],
                                             func=AF.Sqrt, bias=tok[0:126],
                                             accum_out=acc[:, b:b+1])
                    else:
                        for half in range(2):
                            nc.scalar.activation(
                                out=q[:, half*1024:half*1024+1024],
                                in_=src[half][:], func=AF.Sqrt,
                                bias=tok[0:126],
                                accum_out=acc[:, b+half:b+half+1])
                        # pairs 0..2 + pair 3's halves: out as soon as ready
                        nc.sync.dma_start(out=outh[:, 0:B_LOC+1],
                                          in_=acc[:, 0:B_LOC+1])
                # ragged: pair b's partial lands at acc rows 21b, col B_LOC+2
                qr = qpool.tile([84, RAG_F], f32, tag="qrag")
                nc.scalar.activation(out=qr[:], in_=srag[:], func=AF.Sqrt,
                                     bias=tok[0:84],
                                     accum_out=acc[0:84, B_LOC+2:B_LOC+3])
                nc.sync.dma_start(out=outh[0:84, B_LOC+2:B_LOC+3],
                                  in_=acc[0:84, B_LOC+2:B_LOC+3])

            if variant != "full":
                nc.sync.dma_start(out=outh[:, :], in_=acc[:])

            if reps > 1:
                loop_cm.__exit__(None, None, None)

    nc.compile()
    return nc


def _get_module(reps=1):
    key = f"nc{reps}"
    if key not in _CACHE:
        _CACHE[key] = _build_module(reps)
    return _CACHE[key]


def make_in_maps(img1, img2):
    img1 = np.asarray(img1)
    img2 = np.asarray(img2)
    w1, w2, w3all, w3rag = _build_weights()
    in_maps = []
    for d in range(N_CORES):
        sl = slice(d * B_LOC, (d + 1) * B_LOC)
        m = {"w1": w1, "w1f32": w1, "w2": w2, "w3all": w3all,
             "w3rag": w3rag}
        pad = np.full((B_LOC, 2, C, PADPX), 0.5, np.float32)
        pad[:, 0, :, :HWPX] = img1[sl].reshape(B_LOC, C, HWPX)
        pad[:, 1, :, :HWPX] = img2[sl].reshape(B_LOC, C, HWPX)
        m["imgs"] = pad
        in_maps.append(m)
    return in_maps


def kernel(img1, img2):
    import concourse.bass_utils as bass_utils

    img1 = np.ascontiguousarray(np.asarray(img1), dtype=np.float32)
    img2 = np.ascontiguousarray(np.asarray(img2), dtype=np.float32)
    assert img1.shape == (B, C, H, W)

    nc = _get_module()
    in_maps = make_in_maps(img1, img2)

    res = bass_utils.run_bass_kernel_spmd(nc, in_maps, core_ids=list(range(N_CORES)))
    _CACHE["last_results"] = res

    out = np.empty(B, dtype=np.float32)
    for d in range(N_CORES):
        acc = res.results[d]["partials"].astype(np.float64)  # [126, B_LOC+3]
        for b in range(B_LOC):
            if b < B_LOC - 1:
                main = acc[:, b].sum()
            else:
                main = acc[:, b].sum() + acc[:, b + 1].sum()
            rag = acc[21*b:21*b+ROWS, B_LOC + 2].sum()
            out[d * B_LOC + b] = (main + rag) / HWPX
    return out


if __name__ == "__main__":
    i1 = np.load("/root/problem/img1.npy")
    i2 = np.load("/root/problem/img2.npy")
    print(kernel(i1, i2))


# revision 79
# speedup vs baseline: 1.0018x; 1.0018x over previous
"""CIEDE-base color-difference loss kernel for 8 Trainium2 NeuronCores.

Math (lightness_weight = 0, so L never matters):
  lin  = srgb_gamma(x)            -- ACT Ln+Exp, pow branch only; the linear
                                     branch is skipped (~1e-4 rel impact)
  t    = (RGB2XYZ/white) @ lin    -- PE block-diag float32r matmul
  f    = cbrt(t) = exp(ln(t)/3)   -- ACT Ln (from PSUM) + Exp; bf16 out
  da   = 500*((fx1-fy1)-(fx2-fy2));  db = 200*((fy1-fz1)-(fy2-fz2))  -- PE
  s    = da^2 + db^2              -- DVE squares (bf16) + PE pair-sum, packed
                                     at partition rows 21*lg of [126,1024]x2
                                     PSUM accumulators via shifted W3 blocks
  cd   = sqrt(s)                  -- single-pass AF.Sqrt at the very end
  out  = mean over pixels         -- ACT accum_out + host-side f64 reduce

Layout: batch data-parallel, 4 image-pairs per core. Each image plane is
host-padded to 21 partition-rows x 12544 cols (pad = 0.5 in both images so
padded pixels contribute cd ~ 0). Partitions are channel-blocked (p = 21*c+k,
c in r1,g1,b1,r2,g2,b2 -- host packs both images per pair contiguously), so
each [126, F] tile holds 21*F pixel pairs in ONE 2-dim DMA, and every
cross-channel op is a kron(A, I21) matmul on the mostly-idle PE.

Schedule notes (ACT is the bottleneck engine; sim has it ~96% busy):
 - activation-table thrash fix: all ln/exp resolve to one table set (see
   _build_module), saving ~72 reloads (~92us, 28% of the original runtime)
 - 4096-col tiles (2 logical 2048 groups) amortize ACT instruction overhead;
   the first tile's DMA+Ln are eighth-split so ACT starts at ~4us
 - one-step software pipeline: tile t's gamma is emitted before tile t-1's
   cbrt/stage-2/3, so the f32r stage-1 matmuls (1.2GHz) get a full gamma
   pass of head start and the cbrt-Ln chunk reads never starve
 - all sqrts run at the end as single-pass AF.Sqrt behind ONE table switch,
   reading SBUF-parked bf16 s (pairs 0-2) or PSUM directly (pair 3); a
   zero-valued token written by a Relu(scale=0) of the last f tile is used
   as every Sqrt's bias AP, pinning ln/exp before sqrt in the ACT order
   without any cross-engine barrier
 - the pairs' ragged 256-col tails are batched into one [126,1024] chain
   processed LAST (short drain), its input DMA'd late in the stream
Measured on HW (axon): rel_err 7.5e-5; TimelineSim (which matched the
baseline's graded time to 0.4%): 214,988 ns vs 328,174 ns baseline.
"""

import numpy as np
import ml_dtypes

B, C, H, W = 32, 3, 512, 512
HWPX = H * W                 # 262144 pixels per image
N_CORES = 8
B_LOC = B // N_CORES         # 4 image-pairs per core
ROWS = 21                    # partition-rows per image
ROWL = 12544                 # cols per partition-row; host pads each plane to
PADPX = ROWS * ROWL          # 263424 px (+1280 pad px, value 0.5 both images)
FULL_F = 2048                # cols per logical group (spack packing unit)
N_FULL = 6                   # logical groups per image pair (6*2048 = 12288)
TILE_F = 4096                # cols per physical x tile (2 logical groups):
N_TILE = 3                   # fatter ACT instructions amortize fixed overhead
RAG_F = ROWL - N_FULL * FULL_F   # 256 ragged cols

# deg-8 fit of ((x+0.055)/1.055)**2.4 on x in [0,1], evaluated in z = 2x-1
# (f32 fused-Horner max abs err 5.3e-6; used by the DVE-chain gamma path,
# whose exact op sequence is HW-validated bit-for-bit by probe2.py)
_POLY_Z = [0.2140408754348755, 0.4627854526042938, 0.2918641269207001,
           0.03514162823557854, -0.004844842944294214, 0.0010735071264207363,
           -0.0002527319302316755, 0.0005788140115328133,
           -0.0003889543586410582]

_RGB2XYZ = np.array([[0.4124564, 0.3575761, 0.1804375],
                     [0.2126729, 0.7151522, 0.0721750],
                     [0.0193339, 0.1191920, 0.9503041]], dtype=np.float64)
_WHITE = np.array([0.95047, 1.0, 1.08883], dtype=np.float64)

bf16 = ml_dtypes.bfloat16


def _build_weights():
    """Channel-blocked layout: partition p = 21*c + k (c = channel slot 0..5
    meaning r1,g1,b1,r2,g2,b2; k = pixel row 0..20). All block-diagonal maps
    become kron(A, I21)."""
    f32 = np.float32
    I21 = np.eye(ROWS, dtype=f32)
    Mp = (_RGB2XYZ / _WHITE[:, None]).astype(f32)          # 3x3, white folded in
    M6 = np.zeros((6, 6), f32)
    M6[:3, :3] = Mp
    M6[3:, 3:] = Mp
    # stage 1: out = lhsT.T @ rhs; want out[21c'+k] = sum_c M6[c',c] v[21c+k]
    # Used as float32r (full-precision weights)
    W1 = np.kron(M6.T, I21).astype(f32)                     # [126, 126]
    # stage 2: da rows 0..20, db rows 21..41; coeffs exact in bf16
    A = np.array([[500.0, -500.0, 0.0, -500.0, 500.0, 0.0],
                  [0.0, 200.0, -200.0, 0.0, -200.0, 200.0]], f32)
    W2 = np.kron(A.T, I21).astype(f32)                      # [126, 42]
    # stage 3: s[k] = sq[k] + sq[21+k]. For full groups the result is placed
    # at partition rows 21*gg of a [126, F] PSUM accumulator: W3all[:, 126g+p]
    # has the summing I21 block at rows offset 21*g. The cross-pair ragged
    # batch uses B_LOC variants [42, 84] that place pair b's sums at rows 21b.
    W3 = np.kron(np.ones((2, 1), f32), I21).astype(f32)     # [42, 21]
    W3all = np.zeros((42, 126 * N_FULL), f32)
    for g in range(N_FULL):
        W3all[:, 126 * g + 21 * g: 126 * g + 21 * g + ROWS] = W3
    W3rag = np.zeros((42, 84 * B_LOC), f32)
    for b in range(B_LOC):
        W3rag[:, 84 * b + 21 * b: 84 * b + 21 * b + ROWS] = W3
    return W1, W2.astype(bf16), W3all.astype(bf16), W3rag.astype(bf16)


_CACHE = {}


def _build_module(reps=1, variant="full"):
    """variant: 'full' | 'dma' (loads + tiny DVE reduce) | 'dma_act' (loads +
    2 big ACT passes, no PE/DVE pipeline)."""
    import concourse.bass as bass
    import concourse.bacc as bacc
    import concourse.tile as tile
    from concourse import mybir

    # The act-table-load placement pass picks the FIRST table set containing
    # each activation function: Exp -> exp_and_others (id 0, no ln), Ln ->
    # natural_log (id 5, no exp). Alternating Ln/Exp then reloads tables
    # ~72x (~1.3us each, ~92us = 28% of runtime). Hide ln/exp in every set
    # except natural_log_exp_and_others (which genuinely holds both) so the
    # analysis settles on that one set and the load hoists out of the loop.
    # Only placement is affected; the emitted set id stays a valid
    # act_info.json index whose tables contain every function we use.
    if not getattr(bacc, "_lnexp_tables_patch", False):
        _orig_gat = bacc.get_activation_tables

        def _gat_one_set(arch):
            AF = mybir.ActivationFunctionType
            out = {}
            for name, funcs in _orig_gat(arch).items():
                if name != "natural_log_exp_and_others":
                    funcs = funcs - {AF.Ln, AF.Exp}
                out[name] = funcs
            return out

        bacc.get_activation_tables = _gat_one_set
        bacc._lnexp_tables_patch = True

    f32 = mybir.dt.float32
    bft = mybir.dt.bfloat16
    AF = mybir.ActivationFunctionType

    nc = bacc.Bacc(None, target_bir_lowering=False)

    # img1/img2 host-packed per pair: [pair, img, ch, px] makes the full
    # 126-partition block (2*3*21 rows) one uniformly-strided dim, so a
    # [126, F] tile is ONE 2-dim DMA -- halves the HWDGE issue serialization
    imgsh = nc.dram_tensor("imgs", [B_LOC, 2, C, PADPX], f32,
                           kind="ExternalInput")
    f32r = mybir.dt.float32r
    w1h = nc.dram_tensor("w1", [126, 126], f32r, kind="ExternalInput")
    w2h = nc.dram_tensor("w2", [126, 42], bft, kind="ExternalInput")
    w3ah = nc.dram_tensor("w3all", [42, 126 * N_FULL], bft, kind="ExternalInput")
    w3rh = nc.dram_tensor("w3rag", [42, 84 * B_LOC], bft, kind="ExternalInput")
    outh = nc.dram_tensor("partials", [126, B_LOC + 3], f32, kind="ExternalOutput")

    # ln((x+0.055)/1.055) then exp(2.4*l)
    GAMMA_SCALE = float(1.0 / 1.055)
    GAMMA_BIAS = float(0.055 / 1.055)

    def dram_src6(b, col0, ncols):
        """[2 imgs x 3 channels x 21 rows, ncols] of pair b: one DMA."""
        off = b * 2 * C * PADPX + col0
        return bass.AP(tensor=imgsh, offset=off,
                       ap=[[ROWL, 126], [1, ncols]])

    def dram_src_rag():
        """All pairs' trailing RAG_F cols: [126, B_LOC*RAG_F], pair-major
        cols, in one DMA."""
        return bass.AP(tensor=imgsh, offset=N_FULL * FULL_F,
                       ap=[[ROWL, 126], [2 * C * PADPX, B_LOC], [1, RAG_F]])

    with tile.TileContext(nc) as tc:
        from contextlib import ExitStack
        with ExitStack() as ctx:
            singles = ctx.enter_context(tc.tile_pool(name="singles", bufs=1))
            xpool = ctx.enter_context(tc.tile_pool(name="x", bufs=2))
            linpool = ctx.enter_context(tc.tile_pool(name="lin", bufs=4))
            ltpool = ctx.enter_context(tc.tile_pool(name="lt", bufs=1))
            fpool = ctx.enter_context(tc.tile_pool(name="f", bufs=2))
            sqpool = ctx.enter_context(tc.tile_pool(name="sq", bufs=2))
            ddcpool = ctx.enter_context(tc.tile_pool(name="ddc", bufs=2))
            qpool = ctx.enter_context(tc.tile_pool(name="q", bufs=1))
            # PSUM bank budget (8 banks): t [126,512]x3 = 3, dd [42,512] = 1,
            # s [126,1024]x2x2 = 4 (two half-accumulators; srag shares dd)
            tpool = ctx.enter_context(tc.tile_pool(name="t", bufs=3, space="PSUM"))
            ddpool = ctx.enter_context(tc.tile_pool(name="dd", bufs=1, space="PSUM"))
            spool = ctx.enter_context(tc.tile_pool(name="s", bufs=2, space="PSUM"))

            w1f = singles.tile([126, 126], f32r)
            w2 = singles.tile([126, 42], bft)
            w3a = singles.tile([42, 126 * N_FULL], bft)
            w3r = singles.tile([42, 84 * B_LOC], bft)

            # DMA transfers serialize at ~350GB/s in issue order, so issue
            # exactly what the pipeline needs first: the first tile's
            # quarter-splits (emitted in the pair loop below), then weights,
            # then the ragged batch input (xr: computed LAST, parked in a
            # static slot so xpool keeps all its lookahead).
            FR = B_LOC * RAG_F
            xr = singles.tile([126, FR], f32)

            def load_statics():
                nc.sync.dma_start(out=w1f[:], in_=w1h[:, :])
                nc.sync.dma_start(out=w2[:], in_=w2h[:, :])
                nc.sync.dma_start(out=w3a[:], in_=w3ah[:, :])
                nc.sync.dma_start(out=w3r[:], in_=w3rh[:, :])

            # acc cols: 0..2 pairs 0-2, 3/4 pair-3 halves (accum_out overwrites,
            # so the two PSUM-read Sqrts need distinct columns), 5 ragged
            acc = singles.tile([126, B_LOC + 3], f32)
            nc.vector.memset(acc[:], 0.0)

            gbias = singles.tile([128, 1], f32)
            nc.vector.memset(gbias[:], GAMMA_BIAS)
            ebias = singles.tile([128, 1], f32)
            nc.vector.memset(ebias[:], 1e-35)

            # 1-col warmup: hoists the ln/exp table load to t~0.4us; without
            # it the load sits behind the first gamma Ln's DMA-wait (~4us)
            warm = singles.tile([128, 1], f32)
            nc.scalar.activation(out=warm[:], in_=gbias[:], func=AF.Exp)

            # static input/output for the DVE-chain gamma of tile (1,1):
            # z in place on zdve, fused Horner accumulates in lindve
            zdve = singles.tile([126, TILE_F], f32)
            lindve = singles.tile([126, TILE_F], f32r)

            def make_dve_chain():
                """10 closures, each one DVE op of the deg-8 fused Horner
                gamma for tile (1,1); popped 2 per pipeline step so the
                in-order DVE queue never blocks stage-2/3 work for long.
                Replaces the tile's ACT Ln+Exp (~7.2us of the bottleneck
                engine) with ~44us of otherwise-idle DVE time."""
                mul = mybir.AluOpType.mult
                add = mybir.AluOpType.add
                ops = [lambda: nc.vector.tensor_scalar(
                    out=zdve[:], in0=zdve[:], scalar1=2.0, scalar2=-1.0,
                    op0=mul, op1=add)]
                ops.append(lambda: nc.vector.tensor_scalar(
                    out=lindve[:], in0=zdve[:], scalar1=float(_POLY_Z[8]),
                    scalar2=None, op0=mul))
                for kk in range(7, 0, -1):
                    ops.append(lambda kk=kk: nc.vector.scalar_tensor_tensor(
                        out=lindve[:], in0=lindve[:],
                        scalar=float(_POLY_Z[kk]),
                        in1=zdve[:], op0=add, op1=mul))
                ops.append(lambda: nc.vector.tensor_scalar(
                    out=lindve[:], in0=lindve[:], scalar1=float(_POLY_Z[0]),
                    scalar2=None, op0=add))
                return ops


            if reps > 1:
                loop_cm = tc.For_i(0, reps, 1)
                loop_cm.__enter__()

            def gamma(x, F, split_ln=False, ln_dst=None):
                """gamma on a [126, F] tile; returns lin (ACT Ln+Exp only).

                ln_dst: where the gamma Ln lands (default: in place on x).
                The reps>1 timing loop re-runs the body on the SAME parked
                ragged tile, so that caller must keep x pristine -- ln of an
                already-ln'd (negative) value is NaN, and NaN operands are
                catastrophically slow on real hardware."""
                if ln_dst is None:
                    ln_dst = x
                # l = ln((x+0.055)/1.055), lin = exp(2.4 l)
                if split_ln:
                    # per-eighth Ln tracks the eighth-split first DMA
                    for qq in range(F // 512):
                        nc.scalar.activation(
                            out=ln_dst[:, qq*512:qq*512+512],
                            in_=x[:, qq*512:qq*512+512], func=AF.Ln,
                            scale=GAMMA_SCALE, bias=gbias[0:126])
                else:
                    nc.scalar.activation(out=ln_dst[:], in_=x[:], func=AF.Ln,
                                         scale=GAMMA_SCALE, bias=gbias[0:126])
                lin = linpool.tile([126, F], f32r, tag="lin")
                nc.scalar.activation(out=lin[:], in_=ln_dst[:], func=AF.Exp,
                                     scale=2.4)
                return lin

            def cbrt_evac(lin, F, w1sel=None):
                """XYZ stage-1 matmuls + cbrt for a lin tile; returns f.

                Runs one pipeline step behind gamma(): the stage-1 f32r
                matmuls (788ns/512-chunk at the PE's 1.2GHz fp32 rate) get a
                full gamma-pass head start, so the 612ns cbrt-Ln chunk reads
                never drain the 3-buffer tq pool dry."""
                lt = ltpool.tile([126, F], f32, tag="lt")
                for h in range((F + 511) // 512):
                    c0 = h * 512
                    cw = min(512, F - c0)
                    tq = tpool.tile([126, cw], f32, tag="t")
                    w1ap = w1sel if w1sel is not None else w1f[:]
                    nc.tensor.matmul(tq[:], w1ap, lin[:, c0:c0+cw],
                                     start=True, stop=True)
                    # cbrt part 1: lt = ln(t) straight from PSUM
                    nc.scalar.activation(out=lt[:, c0:c0+cw], in_=tq[:],
                                         func=AF.Ln)
                # cbrt part 2: f = exp(lt/3) as bf16
                f = fpool.tile([126, F], bft, tag="f")
                nc.scalar.activation(out=f[:], in_=lt[:], func=AF.Exp,
                                     scale=float(1.0 / 3.0))
                return f

            def stage2_sq(f, F):
                """da,db + squares for a [126, F] f tile; returns sq [42, F]."""
                sq = sqpool.tile([42, F], bft, tag="sq")
                for j in range((F + 511) // 512):
                    s0 = j * 512
                    sw = min(512, F - s0)
                    dd = ddpool.tile([42, sw], f32, tag="dd")
                    nc.tensor.matmul(dd[:], w2[:], f[:, s0:s0+sw],
                                     start=True, stop=True)
                    # DVE tensor_tensor may read only one PSUM operand:
                    # bounce da/db to SBUF bf16, square there (2x mode)
                    ddc = ddcpool.tile([42, sw], bft, tag="ddc")
                    nc.vector.tensor_copy(ddc[:], dd[:])
                    nc.vector.tensor_mul(sq[:, s0:s0+sw], ddc[:], ddc[:])
                return sq

            # ---- main pairs, 6 full groups each. Each pair's packed s is
            # parked to SBUF as bf16 by DVE (idle capacity) right after its
            # last stage-3 matmul; ALL sqrts run at the very end as single-
            # pass AF.Sqrt instructions behind one table switch, instead of
            # per-pair exp(0.5*ln(s)) chains (~7.5us less ACT busy).
            parks = []
            spacks = {}

            def get_spack(b):
                # lazily created by the FIRST s23 of pair b, so spool slot
                # allocation order matches actual write order under the
                # one-step software pipeline
                if b not in spacks:
                    # Two [126, 1024] PSUM accumulators (chunk-halves): rows
                    # 21*lg collect logical group lg's pair-sums via the
                    # shifted W3all blocks (PE writes all 126 partitions;
                    # non-block rows add zero)
                    spacks[b] = [spool.tile([126, FULL_F // 2], f32, tag="s",
                                            name=f"spack{b}_{_h}")
                                 for _h in range(2)]
                return spacks[b]

            s23_done = {}

            def s23(f, F, tt, b, stop_lg):
                """stage 2/3 + (after the pair's last tile) park/collect.
                stop_lg: the logical group accumulated LAST in time for this
                pair (3 when the pool tile's evac is deferred, else 5)."""
                sq = stage2_sq(f, F)
                spack = get_spack(b)
                for j in range(F // 512):
                    s0 = j * 512
                    lg = 2 * tt + (j >= 4)       # logical 2048-col group
                    half, hj = (j % 4) // 2, j % 2
                    nc.tensor.matmul(
                        spack[half][:, hj*512:hj*512+512],
                        w3a[:, 126*lg:126*lg+126],
                        sq[:, s0:s0+512],
                        start=(lg == 0),
                        stop=(lg == stop_lg))
                s23_done[b] = s23_done.get(b, 0) + 1
                if s23_done[b] == N_TILE:
                    if b < B_LOC - 1:
                        # park packed s to SBUF bf16 for the end-of-kernel
                        # Sqrt batch
                        park = singles.tile([126, FULL_F], bft,
                                            name=f"park{b}")
                        for half in range(2):
                            nc.vector.tensor_copy(
                                park[:, half*1024:half*1024+1024],
                                spack[half][:])
                        parks.append(("sbuf", park, b))
                    else:
                        parks.append(("psum", spack, b))

            # ---- main pairs, 3 tiles (6 logical groups) each, software-
            # pipelined one step: tile t's gamma is emitted BEFORE tile
            # t-1's cbrt/stage-2/3, giving the PE a full gamma-pass head
            # start on the stage-1 matmuls. Each pair's packed s is parked
            # to SBUF bf16 by DVE right after its last stage-3 matmul; ALL
            # sqrts run at the very end as single-pass AF.Sqrt instructions
            # behind one table switch (~7.5us less ACT busy than per-pair
            # exp(0.5*ln(s)) chains).
            pending = []
            dve_ops = make_dve_chain()

            def flush_pending(kind=None):
                for ent in list(pending):
                    if kind is not None and ent[-2] != kind:
                        continue
                    ent[-1] -= 1
                    if ent[-1] <= 0:
                        p_lin, p_F, p_tt, p_b, p_stop = ent[:5]
                        s23(cbrt_evac(p_lin, p_F), p_F, p_tt, p_b, p_stop)
                        pending.remove(ent)

            for b in range(B_LOC):
                for tt in range(N_TILE):
                    F = TILE_F
                    col0 = tt * TILE_F

                    k = 3 * b + tt
                    if variant == "full" and k == 4:
                        # tile (1,1) is covered by the DVE chain: no x load,
                        # no ACT gamma; run the remaining chain ops and let
                        # the pipeline flush (chain evac comes at k=5, AFTER
                        # (1,0)'s s23 -- flag order must stay lg 0,1/2,3/4,5
                        # or the start=True of a later-emitted lg0 matmul
                        # wipes the accumulation)
                        flush_pending()
                        for op in dve_ops:
                            op()
                        dve_ops = []
                        continue
                    x = xpool.tile([126, F], f32, tag="x")
                    if b == 0 and tt == 0:
                        # eighth-split the very first load: 512-col transfers
                        # (717ns) pace 512-col Lns (612ns) almost 1:1, so the
                        # pipeline starts after 0.25MB with no startup bubbles
                        for qq in range(8):
                            nc.sync.dma_start(
                                out=x[:, qq*512:qq*512+512],
                                in_=dram_src6(b, col0 + qq*512, 512))
                        load_statics()
                    else:
                        nc.sync.dma_start(out=x[:], in_=dram_src6(b, col0, F))

                    if variant == "dma":
                        red = qpool.tile([126, 1], f32, tag="red")
                        nc.vector.tensor_reduce(
                            out=red[:], in_=x[:], op=mybir.AluOpType.max,
                            axis=mybir.AxisListType.X)
                        continue  # noqa
                    if variant == "dma_act":
                        nc.scalar.activation(out=x[:], in_=x[:], func=AF.Ln,
                                             scale=GAMMA_SCALE, bias=gbias[0:126])
                        nc.scalar.activation(out=x[:], in_=x[:], func=AF.Exp,
                                             scale=2.4)
                        nc.scalar.activation(out=x[:], in_=x[:], func=AF.Ln,
                                             scale=GAMMA_SCALE, bias=gbias[0:126])
                        nc.scalar.activation(out=x[:], in_=x[:], func=AF.Exp,
                                             scale=float(1.0/3.0),
                                             accum_out=acc[0:126, b:b+1])
                        continue

                    if b == B_LOC - 1 and tt == 0:
                        # ragged batch input: needed only at the very end,
                        # so it rides late in the DMA stream where it can't
                        # delay any group tile
                        nc.sync.dma_start(out=xr[:], in_=dram_src_rag())
                    if variant == "full" and k == 1:
                        # tile (1,1)'s data, right after x(0,1) in the sync
                        # stream: a 2MB transfer any earlier steals the
                        # bandwidth the startup eighth-splits depend on.
                        # Emitted before the first chain op (popped below)
                        # so the op orders after the write. Counter 5:
                        # this iteration's flush already decrements it, so
                        # the evac lands at k=5, after (1,0)'s s23.
                        nc.sync.dma_start(out=zdve[:],
                                          in_=dram_src6(1, TILE_F, TILE_F))
                        pending.append([lindve, TILE_F, 1, 1, 5, "main", 5])
                    lin = gamma(x, F, split_ln=(b == 0 and tt == 0))
                    flush_pending()
                    pending.append([lin, F, tt, b, 5, "main", 1])
                    if variant == "full" and dve_ops and k >= 1:
                        for op in dve_ops[:2]:
                            op()
                        dve_ops = dve_ops[2:]

            # ---- cross-pair ragged batch: the pairs' trailing RAG_F cols,
            # packed as [126, B_LOC*RAG_F] (pair b at cols b*RAG_F). Processed
            # LAST: its 2-chunk pipeline drain + [84,256] sqrt make a ~3us
            # tail, vs ~7us for a full tile's 8-chunk drain. The last main
            # tile's cbrt + pair-3 sqrt hide inside the ragged chain.
            if variant == "dma":
                red = qpool.tile([126, 1], f32, tag="red")
                nc.vector.tensor_reduce(
                    out=red[:], in_=xr[:], op=mybir.AluOpType.max,
                    axis=mybir.AxisListType.X)
            elif variant == "dma_act":
                xl = xpool.tile([126, FR], f32, tag="x")
                nc.scalar.activation(out=xl[:], in_=xr[:], func=AF.Ln,
                                     scale=GAMMA_SCALE, bias=gbias[0:126])
                nc.scalar.activation(out=xl[:], in_=xl[:], func=AF.Exp,
                                     scale=2.4,
                                     accum_out=acc[0:126, B_LOC+2:B_LOC+3])
            else:
                xl = xpool.tile([126, FR], f32, tag="x")
                # flush the last main tile BEFORE the ragged gamma: its
                # square/stage-3 drain then overlaps the ragged ACT chain
                while pending:
                    flush_pending()
                lin_r = gamma(xr, FR, ln_dst=xl)
                fr = cbrt_evac(lin_r, FR)
                # ACT-order pin without a barrier: tok = relu(0*f_rag) = 0,
                # exactly zero but data-dependent on the LAST exp; every
                # Sqrt below uses tok as its bias AP (they read a const-0
                # bias AP anyway), so no sqrt can be scheduled before any
                # ln/exp -- one table switch, and no cross-engine gating
                tok = singles.tile([126, 1], f32)
                nc.scalar.activation(out=tok[:], in_=fr[:, 0:1],
                                     func=AF.Relu, scale=0.0)
                sqr = stage2_sq(fr, FR)
                srag = ddpool.tile([84, RAG_F], f32, tag="dd")
                for b in range(B_LOC):
                    nc.tensor.matmul(srag[:], w3r[:, 84*b:84*b+84],
                                     sqr[:, b*RAG_F:(b+1)*RAG_F],
                                     start=(b == 0), stop=(b == B_LOC - 1))

                # ---- all sqrts, single-pass AF.Sqrt behind one table switch.
                # Pairs 0..2 read parked SBUF bf16; pair 3 + ragged read PSUM.
                # The scheduler-only fence keeps every ln/exp before every
                # sqrt in the final ACT order: without it the list scheduler
                # interleaves them and the table-load pass inserts 2 extra
                # ~1.3us reloads.
                for kind, src, b in parks:
                    q = qpool.tile([126, FULL_F], bft, tag="qpack")
                    if kind == "sbuf":
                        nc.scalar.activation(out=q[:], in_=src[:],
                                             func=AF.Sqrt, bias=tok[0:126],
                                             accum_out=acc[:, b:b+1])
                    else:
                        for half in range(2):
                            nc.scalar.activation(
                                out=q[:, half*1024:half*1024+1024],
                                in_=src[half][:], func=AF.Sqrt,
                                bias=tok[0:126],
                                accum_out=acc[:, b+half:b+half+1])
                        # pairs 0..2 + pair 3's halves: out as soon as ready
                        nc.sync.dma_start(out=outh[:, 0:B_LOC+1],
                                          in_=acc[:, 0:B_LOC+1])
                # ragged: pair b's partial lands at acc rows 21b, col B_LOC+2
                qr = qpool.tile([84, RAG_F], f32, tag="qrag")
                nc.scalar.activation(out=qr[:], in_=srag[:], func=AF.Sqrt,
                                     bias=tok[0:84],
                                     accum_out=acc[0:84, B_LOC+2:B_LOC+3])
                nc.sync.dma_start(out=outh[0:84, B_LOC+2:B_LOC+3],
                                  in_=acc[0:84, B_LOC+2:B_LOC+3])

            if variant != "full":
                nc.sync.dma_start(out=outh[:, :], in_=acc[:])

            if reps > 1:
                loop_cm.__exit__(None, None, None)

    nc.compile()
    return nc


def _get_module(reps=1):
    key = f"nc{reps}"
    if key not in _CACHE:
        _CACHE[key] = _build_module(reps)
    return _CACHE[key]


def make_in_maps(img1, img2):
    img1 = np.asarray(img1)
    img2 = np.asarray(img2)
    w1, w2, w3all, w3rag = _build_weights()
    in_maps = []
    for d in range(N_CORES):
        sl = slice(d * B_LOC, (d + 1) * B_LOC)
        m = {"w1": w1, "w2": w2, "w3all": w3all, "w3rag": w3rag}
        pad = np.full((B_LOC, 2, C, PADPX), 0.5, np.float32)
        pad[:, 0, :, :HWPX] = img1[sl].reshape(B_LOC, C, HWPX)
        pad[:, 1, :, :HWPX] = img2[sl].reshape(B_LOC, C, HWPX)
        m["imgs"] = pad
        in_maps.append(m)
    return in_maps


def kernel(img1, img2):
    import concourse.bass_utils as bass_utils

    img1 = np.ascontiguousarray(np.asarray(img1), dtype=np.float32)
    img2 = np.ascontiguousarray(np.asarray(img2), dtype=np.float32)
    assert img1.shape == (B, C, H, W)

    nc = _get_module()
    in_maps = make_in_maps(img1, img2)

    res = bass_utils.run_bass_kernel_spmd(nc, in_maps, core_ids=list(range(N_CORES)))
    _CACHE["last_results"] = res

    out = np.empty(B, dtype=np.float32)
    for d in range(N_CORES):
        acc = res.results[d]["partials"].astype(np.float64)  # [126, B_LOC+3]
        for b in range(B_LOC):
            if b < B_LOC - 1:
                main = acc[:, b].sum()
            else:
                main = acc[:, b].sum() + acc[:, b + 1].sum()
            rag = acc[21*b:21*b+ROWS, B_LOC + 2].sum()
            out[d * B_LOC + b] = (main + rag) / HWPX
    return out


if __name__ == "__main__":
    i1 = np.load("/root/problem/img1.npy")
    i2 = np.load("/root/problem/img2.npy")
    print(kernel(i1, i2))


# revision 80
# speedup vs baseline: 1.0172x; 1.0154x over previous
"""CIEDE-base color-difference loss kernel for 8 Trainium2 NeuronCores.

Math (lightness_weight = 0, so L never matters):
  lin  = srgb_gamma(x)            -- ACT Ln+Exp, pow branch only; the linear
                                     branch is skipped (~1e-4 rel impact)
  t    = (RGB2XYZ/white) @ lin    -- PE block-diag float32r matmul
  f    = cbrt(t) = exp(ln(t)/3)   -- ACT Ln (from PSUM) + Exp; bf16 out
  da   = 500*((fx1-fy1)-(fx2-fy2));  db = 200*((fy1-fz1)-(fy2-fz2))  -- PE
  s    = da^2 + db^2              -- DVE squares (bf16) + PE pair-sum, packed
                                     at partition rows 21*lg of [126,1024]x2
                                     PSUM accumulators via shifted W3 blocks
  cd   = sqrt(s)                  -- single-pass AF.Sqrt at the very end
  out  = mean over pixels         -- ACT accum_out + host-side f64 reduce

Layout: batch data-parallel, 4 image-pairs per core. Each image plane is
host-padded to 21 partition-rows x 12544 cols (pad = 0.5 in both images so
padded pixels contribute cd ~ 0). Partitions are channel-blocked (p = 21*c+k,
c in r1,g1,b1,r2,g2,b2 -- host packs both images per pair contiguously), so
each [126, F] tile holds 21*F pixel pairs in ONE 2-dim DMA, and every
cross-channel op is a kron(A, I21) matmul on the mostly-idle PE.

Schedule notes (ACT is the bottleneck engine; sim has it ~96% busy):
 - activation-table thrash fix: all ln/exp resolve to one table set (see
   _build_module), saving ~72 reloads (~92us, 28% of the original runtime)
 - 4096-col tiles (2 logical 2048 groups) amortize ACT instruction overhead;
   the first tile's DMA+Ln are eighth-split so ACT starts at ~4us
 - one-step software pipeline: tile t's gamma is emitted before tile t-1's
   cbrt/stage-2/3, so the f32r stage-1 matmuls (1.2GHz) get a full gamma
   pass of head start and the cbrt-Ln chunk reads never starve
 - all sqrts run at the end as single-pass AF.Sqrt behind ONE table switch,
   reading SBUF-parked bf16 s (pairs 0-2) or PSUM directly (pair 3); a
   zero-valued token written by a Relu(scale=0) of the last f tile is used
   as every Sqrt's bias AP, pinning ln/exp before sqrt in the ACT order
   without any cross-engine barrier
 - the pairs' ragged 256-col tails are batched into one [126,1024] chain
   processed LAST (short drain), its input DMA'd late in the stream
Measured on HW (axon): rel_err 7.5e-5; TimelineSim (which matched the
baseline's graded time to 0.4%): 214,988 ns vs 328,174 ns baseline.
"""

import numpy as np
import ml_dtypes

B, C, H, W = 32, 3, 512, 512
HWPX = H * W                 # 262144 pixels per image
N_CORES = 8
B_LOC = B // N_CORES         # 4 image-pairs per core
ROWS = 21                    # partition-rows per image
ROWL = 12544                 # cols per partition-row; host pads each plane to
PADPX = ROWS * ROWL          # 263424 px (+1280 pad px, value 0.5 both images)
FULL_F = 2048                # cols per logical group (spack packing unit)
N_FULL = 6                   # logical groups per image pair (6*2048 = 12288)
TILE_F = 4096                # cols per physical x tile (2 logical groups):
N_TILE = 3                   # fatter ACT instructions amortize fixed overhead
RAG_F = ROWL - N_FULL * FULL_F   # 256 ragged cols

# deg-8 fit of ((x+0.055)/1.055)**2.4 on x in [0,1], evaluated in z = 2x-1
# (f32 fused-Horner max abs err 5.3e-6; used by the DVE-chain gamma path,
# whose exact op sequence is HW-validated bit-for-bit by probe2.py)
_POLY_Z = [0.2140408754348755, 0.4627854526042938, 0.2918641269207001,
           0.03514162823557854, -0.004844842944294214, 0.0010735071264207363,
           -0.0002527319302316755, 0.0005788140115328133,
           -0.0003889543586410582]

_RGB2XYZ = np.array([[0.4124564, 0.3575761, 0.1804375],
                     [0.2126729, 0.7151522, 0.0721750],
                     [0.0193339, 0.1191920, 0.9503041]], dtype=np.float64)
_WHITE = np.array([0.95047, 1.0, 1.08883], dtype=np.float64)

bf16 = ml_dtypes.bfloat16


def _build_weights():
    """Channel-blocked layout: partition p = 21*c + k (c = channel slot 0..5
    meaning r1,g1,b1,r2,g2,b2; k = pixel row 0..20). All block-diagonal maps
    become kron(A, I21)."""
    f32 = np.float32
    I21 = np.eye(ROWS, dtype=f32)
    Mp = (_RGB2XYZ / _WHITE[:, None]).astype(f32)          # 3x3, white folded in
    M6 = np.zeros((6, 6), f32)
    M6[:3, :3] = Mp
    M6[3:, 3:] = Mp
    # stage 1: out = lhsT.T @ rhs; want out[21c'+k] = sum_c M6[c',c] v[21c+k]
    # Used as float32r (full-precision weights)
    W1 = np.kron(M6.T, I21).astype(f32)                     # [126, 126]
    # stage 2: da rows 0..20, db rows 21..41; coeffs exact in bf16
    A = np.array([[500.0, -500.0, 0.0, -500.0, 500.0, 0.0],
                  [0.0, 200.0, -200.0, 0.0, -200.0, 200.0]], f32)
    W2 = np.kron(A.T, I21).astype(f32)                      # [126, 42]
    # stage 3: s[k] = sq[k] + sq[21+k]. For full groups the result is placed
    # at partition rows 21*gg of a [126, F] PSUM accumulator: W3all[:, 126g+p]
    # has the summing I21 block at rows offset 21*g. The cross-pair ragged
    # batch uses B_LOC variants [42, 84] that place pair b's sums at rows 21b.
    W3 = np.kron(np.ones((2, 1), f32), I21).astype(f32)     # [42, 21]
    W3all = np.zeros((42, 126 * N_FULL), f32)
    for g in range(N_FULL):
        W3all[:, 126 * g + 21 * g: 126 * g + 21 * g + ROWS] = W3
    W3rag = np.zeros((42, 84 * B_LOC), f32)
    for b in range(B_LOC):
        W3rag[:, 84 * b + 21 * b: 84 * b + 21 * b + ROWS] = W3
    return W1, W2.astype(bf16), W3all.astype(bf16), W3rag.astype(bf16)


_CACHE = {}


def _build_module(reps=1, variant="full"):
    """variant: 'full' | 'dma' (loads + tiny DVE reduce) | 'dma_act' (loads +
    2 big ACT passes, no PE/DVE pipeline)."""
    import concourse.bass as bass
    import concourse.bacc as bacc
    import concourse.tile as tile
    from concourse import mybir

    # The act-table-load placement pass picks the FIRST table set containing
    # each activation function: Exp -> exp_and_others (id 0, no ln), Ln ->
    # natural_log (id 5, no exp). Alternating Ln/Exp then reloads tables
    # ~72x (~1.3us each, ~92us = 28% of runtime). Hide ln/exp in every set
    # except natural_log_exp_and_others (which genuinely holds both) so the
    # analysis settles on that one set and the load hoists out of the loop.
    # Only placement is affected; the emitted set id stays a valid
    # act_info.json index whose tables contain every function we use.
    if not getattr(bacc, "_lnexp_tables_patch", False):
        _orig_gat = bacc.get_activation_tables

        def _gat_one_set(arch):
            AF = mybir.ActivationFunctionType
            out = {}
            for name, funcs in _orig_gat(arch).items():
                if name != "natural_log_exp_and_others":
                    funcs = funcs - {AF.Ln, AF.Exp}
                out[name] = funcs
            return out

        bacc.get_activation_tables = _gat_one_set
        bacc._lnexp_tables_patch = True

    f32 = mybir.dt.float32
    bft = mybir.dt.bfloat16
    AF = mybir.ActivationFunctionType

    nc = bacc.Bacc(None, target_bir_lowering=False)

    # img1/img2 host-packed per pair: [pair, img, ch, px] makes the full
    # 126-partition block (2*3*21 rows) one uniformly-strided dim, so a
    # [126, F] tile is ONE 2-dim DMA -- halves the HWDGE issue serialization
    imgsh = nc.dram_tensor("imgs", [B_LOC, 2, C, PADPX], f32,
                           kind="ExternalInput")
    f32r = mybir.dt.float32r
    w1h = nc.dram_tensor("w1", [126, 126], f32r, kind="ExternalInput")
    w2h = nc.dram_tensor("w2", [126, 42], bft, kind="ExternalInput")
    w3ah = nc.dram_tensor("w3all", [42, 126 * N_FULL], bft, kind="ExternalInput")
    w3rh = nc.dram_tensor("w3rag", [42, 84 * B_LOC], bft, kind="ExternalInput")
    outh = nc.dram_tensor("partials", [126, B_LOC + 3], f32, kind="ExternalOutput")

    # ln((x+0.055)/1.055) then exp(2.4*l)
    GAMMA_SCALE = float(1.0 / 1.055)
    GAMMA_BIAS = float(0.055 / 1.055)

    def dram_src6(b, col0, ncols):
        """[2 imgs x 3 channels x 21 rows, ncols] of pair b: one DMA."""
        off = b * 2 * C * PADPX + col0
        return bass.AP(tensor=imgsh, offset=off,
                       ap=[[ROWL, 126], [1, ncols]])

    def dram_src_rag():
        """All pairs' trailing RAG_F cols: [126, B_LOC*RAG_F], pair-major
        cols, in one DMA."""
        return bass.AP(tensor=imgsh, offset=N_FULL * FULL_F,
                       ap=[[ROWL, 126], [2 * C * PADPX, B_LOC], [1, RAG_F]])

    with tile.TileContext(nc) as tc:
        from contextlib import ExitStack
        with ExitStack() as ctx:
            singles = ctx.enter_context(tc.tile_pool(name="singles", bufs=1))
            xpool = ctx.enter_context(tc.tile_pool(name="x", bufs=2))
            linpool = ctx.enter_context(tc.tile_pool(name="lin", bufs=4))
            ltpool = ctx.enter_context(tc.tile_pool(name="lt", bufs=1))
            fpool = ctx.enter_context(tc.tile_pool(name="f", bufs=2))
            sqpool = ctx.enter_context(tc.tile_pool(name="sq", bufs=2))
            ddcpool = ctx.enter_context(tc.tile_pool(name="ddc", bufs=2))
            qpool = ctx.enter_context(tc.tile_pool(name="q", bufs=1))
            # PSUM bank budget (8 banks): t [126,512]x3 = 3, dd [42,512] = 1,
            # s [126,1024]x2x2 = 4 (two half-accumulators; srag shares dd)
            tpool = ctx.enter_context(tc.tile_pool(name="t", bufs=3, space="PSUM"))
            ddpool = ctx.enter_context(tc.tile_pool(name="dd", bufs=1, space="PSUM"))
            spool = ctx.enter_context(tc.tile_pool(name="s", bufs=2, space="PSUM"))

            w1f = singles.tile([126, 126], f32r)
            w2 = singles.tile([126, 42], bft)
            w3a = singles.tile([42, 126 * N_FULL], bft)
            w3r = singles.tile([42, 84 * B_LOC], bft)

            # DMA transfers serialize at ~350GB/s in issue order, so issue
            # exactly what the pipeline needs first: the first tile's
            # quarter-splits (emitted in the pair loop below), then weights,
            # then the ragged batch input (xr: computed LAST, parked in a
            # static slot so xpool keeps all its lookahead).
            FR = B_LOC * RAG_F
            xr = singles.tile([126, FR], f32)

            def load_statics():
                nc.sync.dma_start(out=w1f[:], in_=w1h[:, :])
                nc.sync.dma_start(out=w2[:], in_=w2h[:, :])
                nc.sync.dma_start(out=w3a[:], in_=w3ah[:, :])
                nc.sync.dma_start(out=w3r[:], in_=w3rh[:, :])

            # acc cols: 0..2 pairs 0-2, 3/4 pair-3 halves (accum_out overwrites,
            # so the two PSUM-read Sqrts need distinct columns), 5 ragged
            acc = singles.tile([126, B_LOC + 3], f32)
            nc.vector.memset(acc[:], 0.0)

            gbias = singles.tile([128, 1], f32)
            nc.vector.memset(gbias[:], GAMMA_BIAS)
            ebias = singles.tile([128, 1], f32)
            nc.vector.memset(ebias[:], 1e-35)

            # 1-col warmup: hoists the ln/exp table load to t~0.4us; without
            # it the load sits behind the first gamma Ln's DMA-wait (~4us)
            warm = singles.tile([128, 1], f32)
            nc.scalar.activation(out=warm[:], in_=gbias[:], func=AF.Exp)

            # static input/output for the DVE-chain gamma of tile (1,1):
            # z in place on zdve, fused Horner accumulates in lindve
            zdve = singles.tile([126, TILE_F], f32)
            lindve = singles.tile([126, TILE_F], f32r)

            def make_dve_chain():
                """10 closures, each one DVE op of the deg-8 fused Horner
                gamma for tile (1,1); popped 2 per pipeline step so the
                in-order DVE queue never blocks stage-2/3 work for long.
                Replaces the tile's ACT Ln+Exp (~7.2us of the bottleneck
                engine) with ~44us of otherwise-idle DVE time."""
                mul = mybir.AluOpType.mult
                add = mybir.AluOpType.add
                ops = [lambda: nc.vector.tensor_scalar(
                    out=zdve[:], in0=zdve[:], scalar1=2.0, scalar2=-1.0,
                    op0=mul, op1=add)]
                ops.append(lambda: nc.vector.tensor_scalar(
                    out=lindve[:], in0=zdve[:], scalar1=float(_POLY_Z[8]),
                    scalar2=None, op0=mul))
                for kk in range(7, 0, -1):
                    ops.append(lambda kk=kk: nc.vector.scalar_tensor_tensor(
                        out=lindve[:], in0=lindve[:],
                        scalar=float(_POLY_Z[kk]),
                        in1=zdve[:], op0=add, op1=mul))
                ops.append(lambda: nc.vector.tensor_scalar(
                    out=lindve[:], in0=lindve[:], scalar1=float(_POLY_Z[0]),
                    scalar2=None, op0=add))
                return ops

            def make_dve_half_chain():
                """Same fused Horner on HALF a tile (cols 2048:4096 of
                lindve, z in zdve[:, 0:2048]): 2048-col ops are ~2.25us, so
                2-per-step pops fit beside the stage-2/3 DVE load. The ACT
                half of the hybrid tile writes lindve[:, 0:2048]."""
                mul = mybir.AluOpType.mult
                add = mybir.AluOpType.add
                H = TILE_F // 2
                z = zdve
                ln = lindve
                ops = [lambda: nc.vector.tensor_scalar(
                    out=z[:, 0:H], in0=z[:, 0:H], scalar1=2.0, scalar2=-1.0,
                    op0=mul, op1=add)]
                ops.append(lambda: nc.vector.tensor_scalar(
                    out=ln[:, H:2*H], in0=z[:, 0:H],
                    scalar1=float(_POLY_Z[8]), scalar2=None, op0=mul))
                for kk in range(7, 0, -1):
                    ops.append(lambda kk=kk: nc.vector.scalar_tensor_tensor(
                        out=ln[:, H:2*H], in0=ln[:, H:2*H],
                        scalar=float(_POLY_Z[kk]),
                        in1=z[:, 0:H], op0=add, op1=mul))
                ops.append(lambda: nc.vector.tensor_scalar(
                    out=ln[:, H:2*H], in0=ln[:, H:2*H],
                    scalar1=float(_POLY_Z[0]), scalar2=None, op0=add))
                return ops


            if reps > 1:
                loop_cm = tc.For_i(0, reps, 1)
                loop_cm.__enter__()

            def gamma(x, F, split_ln=False, ln_dst=None):
                """gamma on a [126, F] tile; returns lin (ACT Ln+Exp only).

                ln_dst: where the gamma Ln lands (default: in place on x).
                The reps>1 timing loop re-runs the body on the SAME parked
                ragged tile, so that caller must keep x pristine -- ln of an
                already-ln'd (negative) value is NaN, and NaN operands are
                catastrophically slow on real hardware."""
                if ln_dst is None:
                    ln_dst = x
                # l = ln((x+0.055)/1.055), lin = exp(2.4 l)
                if split_ln:
                    # per-eighth Ln tracks the eighth-split first DMA
                    for qq in range(F // 512):
                        nc.scalar.activation(
                            out=ln_dst[:, qq*512:qq*512+512],
                            in_=x[:, qq*512:qq*512+512], func=AF.Ln,
                            scale=GAMMA_SCALE, bias=gbias[0:126])
                else:
                    nc.scalar.activation(out=ln_dst[:], in_=x[:], func=AF.Ln,
                                         scale=GAMMA_SCALE, bias=gbias[0:126])
                lin = linpool.tile([126, F], f32r, tag="lin")
                nc.scalar.activation(out=lin[:], in_=ln_dst[:], func=AF.Exp,
                                     scale=2.4)
                return lin

            def cbrt_evac(lin, F, w1sel=None):
                """XYZ stage-1 matmuls + cbrt for a lin tile; returns f.

                Runs one pipeline step behind gamma(): the stage-1 f32r
                matmuls (788ns/512-chunk at the PE's 1.2GHz fp32 rate) get a
                full gamma-pass head start, so the 612ns cbrt-Ln chunk reads
                never drain the 3-buffer tq pool dry."""
                lt = ltpool.tile([126, F], f32, tag="lt")
                for h in range((F + 511) // 512):
                    c0 = h * 512
                    cw = min(512, F - c0)
                    tq = tpool.tile([126, cw], f32, tag="t")
                    w1ap = w1sel if w1sel is not None else w1f[:]
                    nc.tensor.matmul(tq[:], w1ap, lin[:, c0:c0+cw],
                                     start=True, stop=True)
                    # cbrt part 1: lt = ln(t) straight from PSUM
                    nc.scalar.activation(out=lt[:, c0:c0+cw], in_=tq[:],
                                         func=AF.Ln)
                # cbrt part 2: f = exp(lt/3) as bf16
                f = fpool.tile([126, F], bft, tag="f")
                nc.scalar.activation(out=f[:], in_=lt[:], func=AF.Exp,
                                     scale=float(1.0 / 3.0))
                return f

            def stage2_sq(f, F):
                """da,db + squares for a [126, F] f tile; returns sq [42, F]."""
                sq = sqpool.tile([42, F], bft, tag="sq")
                for j in range((F + 511) // 512):
                    s0 = j * 512
                    sw = min(512, F - s0)
                    dd = ddpool.tile([42, sw], f32, tag="dd")
                    nc.tensor.matmul(dd[:], w2[:], f[:, s0:s0+sw],
                                     start=True, stop=True)
                    # DVE tensor_tensor may read only one PSUM operand:
                    # bounce da/db to SBUF bf16, square there (2x mode)
                    ddc = ddcpool.tile([42, sw], bft, tag="ddc")
                    nc.vector.tensor_copy(ddc[:], dd[:])
                    nc.vector.tensor_mul(sq[:, s0:s0+sw], ddc[:], ddc[:])
                return sq

            # ---- main pairs, 6 full groups each. Each pair's packed s is
            # parked to SBUF as bf16 by DVE (idle capacity) right after its
            # last stage-3 matmul; ALL sqrts run at the very end as single-
            # pass AF.Sqrt instructions behind one table switch, instead of
            # per-pair exp(0.5*ln(s)) chains (~7.5us less ACT busy).
            parks = []
            spacks = {}

            def get_spack(b):
                # lazily created by the FIRST s23 of pair b, so spool slot
                # allocation order matches actual write order under the
                # one-step software pipeline
                if b not in spacks:
                    # Two [126, 1024] PSUM accumulators (chunk-halves): rows
                    # 21*lg collect logical group lg's pair-sums via the
                    # shifted W3all blocks (PE writes all 126 partitions;
                    # non-block rows add zero)
                    spacks[b] = [spool.tile([126, FULL_F // 2], f32, tag="s",
                                            name=f"spack{b}_{_h}")
                                 for _h in range(2)]
                return spacks[b]

            s23_done = {}

            def s23(f, F, tt, b, stop_lg):
                """stage 2/3 + (after the pair's last tile) park/collect.
                stop_lg: the logical group accumulated LAST in time for this
                pair (3 when the pool tile's evac is deferred, else 5)."""
                sq = stage2_sq(f, F)
                spack = get_spack(b)
                for j in range(F // 512):
                    s0 = j * 512
                    lg = 2 * tt + (j >= 4)       # logical 2048-col group
                    half, hj = (j % 4) // 2, j % 2
                    nc.tensor.matmul(
                        spack[half][:, hj*512:hj*512+512],
                        w3a[:, 126*lg:126*lg+126],
                        sq[:, s0:s0+512],
                        start=(lg == 0),
                        stop=(lg == stop_lg))
                s23_done[b] = s23_done.get(b, 0) + 1
                if s23_done[b] == N_TILE:
                    if b < B_LOC - 1:
                        # park packed s to SBUF bf16 for the end-of-kernel
                        # Sqrt batch
                        park = singles.tile([126, FULL_F], bft,
                                            name=f"park{b}")
                        for half in range(2):
                            nc.vector.tensor_copy(
                                park[:, half*1024:half*1024+1024],
                                spack[half][:])
                        parks.append(("sbuf", park, b))
                    else:
                        parks.append(("psum", spack, b))

            # ---- main pairs, 3 tiles (6 logical groups) each, software-
            # pipelined one step: tile t's gamma is emitted BEFORE tile
            # t-1's cbrt/stage-2/3, giving the PE a full gamma-pass head
            # start on the stage-1 matmuls. Each pair's packed s is parked
            # to SBUF bf16 by DVE right after its last stage-3 matmul; ALL
            # sqrts run at the very end as single-pass AF.Sqrt instructions
            # behind one table switch (~7.5us less ACT busy than per-pair
            # exp(0.5*ln(s)) chains).
            pending = []
            dve_ops = make_dve_chain()
            dve_ops2 = make_dve_half_chain()

            def flush_pending(kind=None):
                for ent in list(pending):
                    if kind is not None and ent[-2] != kind:
                        continue
                    ent[-1] -= 1
                    if ent[-1] <= 0:
                        p_lin, p_F, p_tt, p_b, p_stop = ent[:5]
                        s23(cbrt_evac(p_lin, p_F), p_F, p_tt, p_b, p_stop)
                        pending.remove(ent)

            for b in range(B_LOC):
                for tt in range(N_TILE):
                    F = TILE_F
                    col0 = tt * TILE_F

                    k = 3 * b + tt
                    if variant == "full" and k == 4:
                        # tile (1,1) is covered by the DVE chain: no x load,
                        # no ACT gamma; run the remaining chain ops and let
                        # the pipeline flush (chain evac comes at k=5, AFTER
                        # (1,0)'s s23 -- flag order must stay lg 0,1/2,3/4,5
                        # or the start=True of a later-emitted lg0 matmul
                        # wipes the accumulation)
                        flush_pending()
                        for op in dve_ops:
                            op()
                        dve_ops = []
                        continue
                    x = xpool.tile([126, F], f32, tag="x")
                    if b == 0 and tt == 0:
                        # eighth-split the very first load: 512-col transfers
                        # (717ns) pace 512-col Lns (612ns) almost 1:1, so the
                        # pipeline starts after 0.25MB with no startup bubbles
                        for qq in range(8):
                            nc.sync.dma_start(
                                out=x[:, qq*512:qq*512+512],
                                in_=dram_src6(b, col0 + qq*512, 512))
                        load_statics()
                    elif variant == "full" and k == 10:
                        nc.sync.dma_start(out=x[:, 0:TILE_F // 2],
                                          in_=dram_src6(b, col0,
                                                        TILE_F // 2))
                    else:
                        nc.sync.dma_start(out=x[:], in_=dram_src6(b, col0, F))

                    if variant == "dma":
                        red = qpool.tile([126, 1], f32, tag="red")
                        nc.vector.tensor_reduce(
                            out=red[:], in_=x[:], op=mybir.AluOpType.max,
                            axis=mybir.AxisListType.X)
                        continue  # noqa
                    if variant == "dma_act":
                        nc.scalar.activation(out=x[:], in_=x[:], func=AF.Ln,
                                             scale=GAMMA_SCALE, bias=gbias[0:126])
                        nc.scalar.activation(out=x[:], in_=x[:], func=AF.Exp,
                                             scale=2.4)
                        nc.scalar.activation(out=x[:], in_=x[:], func=AF.Ln,
                                             scale=GAMMA_SCALE, bias=gbias[0:126])
                        nc.scalar.activation(out=x[:], in_=x[:], func=AF.Exp,
                                             scale=float(1.0/3.0),
                                             accum_out=acc[0:126, b:b+1])
                        continue

                    if b == B_LOC - 1 and tt == 0:
                        # ragged batch input: needed only at the very end,
                        # so it rides late in the DMA stream where it can't
                        # delay any group tile
                        nc.sync.dma_start(out=xr[:], in_=dram_src_rag())
                    if variant == "full" and k == 6:
                        # DVE half-chain input for tile (3,1): cols 2048:4096
                        # into zdve[:, 0:2048]; the write naturally orders
                        # after chain 1's last z read
                        nc.sync.dma_start(
                            out=zdve[:, 0:TILE_F // 2],
                            in_=dram_src6(3, TILE_F + TILE_F // 2,
                                          TILE_F // 2))
                    if variant == "full" and k == 1:
                        # tile (1,1)'s data, right after x(0,1) in the sync
                        # stream: a 2MB transfer any earlier steals the
                        # bandwidth the startup eighth-splits depend on.
                        # Emitted before the first chain op (popped below)
                        # so the op orders after the write. Counter 5:
                        # this iteration's flush already decrements it, so
                        # the evac lands at k=5, after (1,0)'s s23.
                        nc.sync.dma_start(out=zdve[:],
                                          in_=dram_src6(1, TILE_F, TILE_F))
                        pending.append([lindve, TILE_F, 1, 1, 5, "main", 5])
                    if variant == "full" and k == 10:
                        # hybrid tile (3,1): ACT gammas cols 0:2048 into
                        # lindve's low half; the DVE half-chain has already
                        # filled the high half. One evac covers both.
                        nc.scalar.activation(out=x[:, 0:TILE_F // 2],
                                             in_=x[:, 0:TILE_F // 2],
                                             func=AF.Ln, scale=GAMMA_SCALE,
                                             bias=gbias[0:126])
                        nc.scalar.activation(out=lindve[:, 0:TILE_F // 2],
                                             in_=x[:, 0:TILE_F // 2],
                                             func=AF.Exp, scale=2.4)
                        lin = lindve
                    else:
                        lin = gamma(x, F, split_ln=(b == 0 and tt == 0))
                    flush_pending()
                    pending.append([lin, F, tt, b, 5, "main", 1])
                    if variant == "full" and dve_ops and k >= 1:
                        for op in dve_ops[:2]:
                            op()
                        dve_ops = dve_ops[2:]
                    if variant == "full" and dve_ops2 and k >= 6:
                        for op in dve_ops2[:2]:
                            op()
                        dve_ops2 = dve_ops2[2:]

            # ---- cross-pair ragged batch: the pairs' trailing RAG_F cols,
            # packed as [126, B_LOC*RAG_F] (pair b at cols b*RAG_F). Processed
            # LAST: its 2-chunk pipeline drain + [84,256] sqrt make a ~3us
            # tail, vs ~7us for a full tile's 8-chunk drain. The last main
            # tile's cbrt + pair-3 sqrt hide inside the ragged chain.
            if variant == "dma":
                red = qpool.tile([126, 1], f32, tag="red")
                nc.vector.tensor_reduce(
                    out=red[:], in_=xr[:], op=mybir.AluOpType.max,
                    axis=mybir.AxisListType.X)
            elif variant == "dma_act":
                xl = xpool.tile([126, FR], f32, tag="x")
                nc.scalar.activation(out=xl[:], in_=xr[:], func=AF.Ln,
                                     scale=GAMMA_SCALE, bias=gbias[0:126])
                nc.scalar.activation(out=xl[:], in_=xl[:], func=AF.Exp,
                                     scale=2.4,
                                     accum_out=acc[0:126, B_LOC+2:B_LOC+3])
            else:
                xl = xpool.tile([126, FR], f32, tag="x")
                # flush the last main tile BEFORE the ragged gamma: its
                # square/stage-3 drain then overlaps the ragged ACT chain
                while pending:
                    flush_pending()
                lin_r = gamma(xr, FR, ln_dst=xl)
                fr = cbrt_evac(lin_r, FR)
                # ACT-order pin without a barrier: tok = relu(0*f_rag) = 0,
                # exactly zero but data-dependent on the LAST exp; every
                # Sqrt below uses tok as its bias AP (they read a const-0
                # bias AP anyway), so no sqrt can be scheduled before any
                # ln/exp -- one table switch, and no cross-engine gating
                tok = singles.tile([126, 1], f32)
                nc.scalar.activation(out=tok[:], in_=fr[:, 0:1],
                                     func=AF.Relu, scale=0.0)
                sqr = stage2_sq(fr, FR)
                srag = ddpool.tile([84, RAG_F], f32, tag="dd")
                for b in range(B_LOC):
                    nc.tensor.matmul(srag[:], w3r[:, 84*b:84*b+84],
                                     sqr[:, b*RAG_F:(b+1)*RAG_F],
                                     start=(b == 0), stop=(b == B_LOC - 1))

                # ---- all sqrts, single-pass AF.Sqrt behind one table switch.
                # Pairs 0..2 read parked SBUF bf16; pair 3 + ragged read PSUM.
                # The scheduler-only fence keeps every ln/exp before every
                # sqrt in the final ACT order: without it the list scheduler
                # interleaves them and the table-load pass inserts 2 extra
                # ~1.3us reloads.
                for kind, src, b in parks:
                    q = qpool.tile([126, FULL_F], bft, tag="qpack")
                    if kind == "sbuf":
                        nc.scalar.activation(out=q[:], in_=src[:],
                                             func=AF.Sqrt, bias=tok[0:126],
                                             accum_out=acc[:, b:b+1])
                    else:
                        for half in range(2):
                            nc.scalar.activation(
                                out=q[:, half*1024:half*1024+1024],
                                in_=src[half][:], func=AF.Sqrt,
                                bias=tok[0:126],
                                accum_out=acc[:, b+half:b+half+1])
                        # pairs 0..2 + pair 3's halves: out as soon as ready
                        nc.sync.dma_start(out=outh[:, 0:B_LOC+1],
                                          in_=acc[:, 0:B_LOC+1])
                # ragged: pair b's partial lands at acc rows 21b, col B_LOC+2
                qr = qpool.tile([84, RAG_F], f32, tag="qrag")
                nc.scalar.activation(out=qr[:], in_=srag[:], func=AF.Sqrt,
                                     bias=tok[0:84],
                                     accum_out=acc[0:84, B_LOC+2:B_LOC+3])
                nc.sync.dma_start(out=outh[0:84, B_LOC+2:B_LOC+3],
                                  in_=acc[0:84, B_LOC+2:B_LOC+3])

            if variant != "full":
                nc.sync.dma_start(out=outh[:, :], in_=acc[:])

            if reps > 1:
                loop_cm.__exit__(None, None, None)

    nc.compile()
    return nc


def _get_module(reps=1):
    key = f"nc{reps}"
    if key not in _CACHE:
        _CACHE[key] = _build_module(reps)
    return _CACHE[key]


def make_in_maps(img1, img2):
    img1 = np.asarray(img1)
    img2 = np.asarray(img2)
    w1, w2, w3all, w3rag = _build_weights()
    in_maps = []
    for d in range(N_CORES):
        sl = slice(d * B_LOC, (d + 1) * B_LOC)
        m = {"w1": w1, "w2": w2, "w3all": w3all, "w3rag": w3rag}
        pad = np.full((B_LOC, 2, C, PADPX), 0.5, np.float32)
        pad[:, 0, :, :HWPX] = img1[sl].reshape(B_LOC, C, HWPX)
        pad[:, 1, :, :HWPX] = img2[sl].reshape(B_LOC, C, HWPX)
        m["imgs"] = pad
        in_maps.append(m)
    return in_maps


def kernel(img1, img2):
    import concourse.bass_utils as bass_utils

    img1 = np.ascontiguousarray(np.asarray(img1), dtype=np.float32)
    img2 = np.ascontiguousarray(np.asarray(img2), dtype=np.float32)
    assert img1.shape == (B, C, H, W)

    nc = _get_module()
    in_maps = make_in_maps(img1, img2)

    res = bass_utils.run_bass_kernel_spmd(nc, in_maps, core_ids=list(range(N_CORES)))
    _CACHE["last_results"] = res

    out = np.empty(B, dtype=np.float32)
    for d in range(N_CORES):
        acc = res.results[d]["partials"].astype(np.float64)  # [126, B_LOC+3]
        for b in range(B_LOC):
            if b < B_LOC - 1:
                main = acc[:, b].sum()
            else:
                main = acc[:, b].sum() + acc[:, b + 1].sum()
            rag = acc[21*b:21*b+ROWS, B_LOC + 2].sum()
            out[d * B_LOC + b] = (main + rag) / HWPX
    return out


if __name__ == "__main__":
    i1 = np.load("/root/problem/img1.npy")
    i2 = np.load("/root/problem/img2.npy")
    print(kernel(i1, i2))
